# revision 5
# baseline (speedup 1.0000x reference)
"""InvertedReorg (depth-to-space, slice=2) Trainium2 Bass kernel.

Full input x: (32, 256, 64, 64) f32 -> output (32, 64, 128, 128) f32 with
    y[b, c, s1*64 + h, s2*64 + w] = x[b, s1*128 + s2*64 + c, h, w]
i.e. the output image is a 2x2 grid of 64x64 blocks, each block one full
input channel map.

Data-parallel over batch: 32 samples / 8 cores = 4 samples per core.

Per-core pipeline (per sample b):
  1. 4 loads HBM->SBUF, one per (s1, s2) channel group of 64 channels.
     Tile T[p = 2c+s1, f = s2*4096 + h*64 + w]; every DMA descriptor is a
     16 KiB contiguous channel map, and the stride-2 partition slice keeps
     all 16 SDMA ports busy.
  2. On-chip shuffle into S[p, f = h*128 + s2*64 + w] - the 256B interleave
     the permutation needs is a pure within-partition strided copy, split
     across the vector and scalar engines (one s2 half each).
  3. 1 store SBUF->HBM: S[:, :] -> y[b] viewed as [(c s1), (h w)]; 128
     descriptors x 32 KiB, fully contiguous on the HBM side.
"""

import numpy as np

_B, _CH, _H, _W = 32, 256, 64, 64
_NCORES = 8
_BPC = _B // _NCORES  # samples per core
_C = _CH // 4  # output channels
_HW = _H * _W  # 4096
_FD = 2 * _HW  # 8192 free-dim elements per partition

_cache = {}


def _split_multiwaits(nc, mybir):
    """This walrus build allows one sync-wait command per instruction.
    Tile attaches one wait per dependency, so split the extras into
    same-engine NoOps directly preceding the instruction (the engine
    blocks on each in turn - semantics unchanged)."""
    for f in nc.m.functions:
        for b in f.blocks:
            new_insts = []
            for inst in b.instructions:
                si = inst.sync_info
                if si is not None and len(si.on_wait) > 1:
                    for w in si.on_wait[:-1]:
                        new_insts.append(
                            mybir.InstNoOp(
                                name=f"I-{nc.next_id()}",
                                engine=inst.engine,
                                ins=[],
                                outs=[],
                                sync_info=mybir.SyncInfo(on_wait=[w], on_update=[]),
                            )
                        )
                    inst.sync_info = mybir.SyncInfo(
                        on_wait=[si.on_wait[-1]], on_update=list(si.on_update)
                    )
                new_insts.append(inst)
            b.instructions = new_insts


def _build():
    from concourse import bass, mybir, tile

    nc = bass.Bass()
    x = nc.declare_dram_parameter(
        "x", [_BPC, _CH, _H, _W], mybir.dt.float32, isOutput=False
    )
    y = nc.declare_dram_parameter(
        "y", [_BPC, _C, 2 * _H, 2 * _W], mybir.dt.float32, isOutput=True
    )
    # Pipeline granularity: half a sample along h (_NH h-rows per chunk),
    # so the store stream starts early and trails the load stream by only
    # half an iteration -- maximizing the dual-stream (load||store) window.
    _NH = _H // 2  # 32 h-rows per chunk
    _CH_FD = _NH * _W  # 2048 elements per (s2) group per partition
    _CFD = 2 * _CH_FD  # 4096 free-dim elements per tile
    # x viewed as [b, s1, s2, c, hh, (q w)] with h = hh*_NH + q
    xr = x.rearrange(
        "b (s1 s2 c) (hh q) w -> b s1 s2 c hh (q w)", s1=2, s2=2, hh=2
    )
    # y viewed as [b, hh, (c s1), (q w)] -- partition p = 2c + s1
    yr = y.rearrange("b c (s hh q) w -> b hh (c s) (q w)", s=2, hh=2)

    with tile.TileContext(nc) as tc:
        with (
            tc.tile_pool(name="tin", bufs=3) as pin,
            tc.tile_pool(name="tout", bufs=3) as pout,
        ):
            for b in range(_BPC):
                for hh in range(2):
                    T = pin.tile([128, _CFD], mybir.dt.float32)
                    for s1 in range(2):
                        for s2 in range(2):
                            nc.sync.dma_start(
                                out=T[s1::2, s2 * _CH_FD : (s2 + 1) * _CH_FD],
                                in_=xr[b, s1, s2, :, hh],
                            )
                    S = pout.tile([128, _CFD], mybir.dt.float32)
                    S3 = S.rearrange("p (h x) -> p h x", x=2 * _W)
                    for s2 in range(2):
                        src = T[:, s2 * _CH_FD : (s2 + 1) * _CH_FD].rearrange(
                            "p (h w) -> p h w", w=_W
                        )
                        dst = S3[:, :, s2 * _W : (s2 + 1) * _W]
                        if s2 == 0:
                            nc.vector.tensor_copy(out=dst, in_=src)
                        else:
                            nc.scalar.copy(out=dst, in_=src)
                    # SWDGE (gpsimd) ring for stores: keeps them off the SP
                    # HWDGE ring so chunk i's store can't head-of-line
                    # block chunk i+1's loads while waiting on copies(i).
                    nc.gpsimd.dma_start(out=yr[b, hh], in_=S[:, :])
    _split_multiwaits(nc, mybir)
    return nc


def kernel(x: np.ndarray) -> np.ndarray:
    from concourse.bass_utils import run_bass_kernel_spmd

    if "nc" not in _cache:
        _cache["nc"] = _build()
    nc = _cache["nc"]

    x = np.ascontiguousarray(np.asarray(x), dtype=np.float32)
    in_maps = [{"x": x[i * _BPC : (i + 1) * _BPC]} for i in range(_NCORES)]
    res = run_bass_kernel_spmd(nc, in_maps, list(range(_NCORES)))
    return np.concatenate([res.results[i]["y"] for i in range(_NCORES)], axis=0)


# revision 6
# speedup vs baseline: 1.1023x; 1.1023x over previous
"""InvertedReorg (depth-to-space, slice=2) Trainium2 Bass kernel.

Full input x: (32, 256, 64, 64) f32 -> output (32, 64, 128, 128) f32 with
    y[b, c, s1*64 + h, s2*64 + w] = x[b, s1*128 + s2*64 + c, h, w]
i.e. the output image is a 2x2 grid of 64x64 blocks, each block one full
input channel map.

Data-parallel over batch: 32 samples / 8 cores = 4 samples per core.

Per-core pipeline (per sample b):
  1. 4 loads HBM->SBUF, one per (s1, s2) channel group of 64 channels.
     Tile T[p = 2c+s1, f = s2*4096 + h*64 + w]; every DMA descriptor is a
     16 KiB contiguous channel map, and the stride-2 partition slice keeps
     all 16 SDMA ports busy.
  2. On-chip shuffle into S[p, f = h*128 + s2*64 + w] - the 256B interleave
     the permutation needs is a pure within-partition strided copy, split
     across the vector and scalar engines (one s2 half each).
  3. 1 store SBUF->HBM: S[:, :] -> y[b] viewed as [(c s1), (h w)]; 128
     descriptors x 32 KiB, fully contiguous on the HBM side.
"""

import numpy as np

_B, _CH, _H, _W = 32, 256, 64, 64
_NCORES = 8
_BPC = _B // _NCORES  # samples per core
_C = _CH // 4  # output channels
_HW = _H * _W  # 4096
_FD = 2 * _HW  # 8192 free-dim elements per partition

_cache = {}


def _split_multiwaits(nc, mybir):
    """This walrus build allows one sync-wait command per instruction.
    Tile attaches one wait per dependency, so split the extras into
    same-engine NoOps directly preceding the instruction (the engine
    blocks on each in turn - semantics unchanged)."""
    for f in nc.m.functions:
        for b in f.blocks:
            new_insts = []
            for inst in b.instructions:
                si = inst.sync_info
                if si is not None and len(si.on_wait) > 1:
                    for w in si.on_wait[:-1]:
                        new_insts.append(
                            mybir.InstNoOp(
                                name=f"I-{nc.next_id()}",
                                engine=inst.engine,
                                ins=[],
                                outs=[],
                                sync_info=mybir.SyncInfo(on_wait=[w], on_update=[]),
                            )
                        )
                    inst.sync_info = mybir.SyncInfo(
                        on_wait=[si.on_wait[-1]], on_update=list(si.on_update)
                    )
                new_insts.append(inst)
            b.instructions = new_insts


def _build():
    from concourse import bass, mybir, tile

    nc = bass.Bass()
    x = nc.declare_dram_parameter(
        "x", [_BPC, _CH, _H, _W], mybir.dt.float32, isOutput=False
    )
    y = nc.declare_dram_parameter(
        "y", [_BPC, _C, 2 * _H, 2 * _W], mybir.dt.float32, isOutput=True
    )
    # x viewed as [b, s1, s2, c, (h w)]
    xr = x.rearrange("b (s1 s2 c) h w -> b s1 s2 c (h w)", s1=2, s2=2)
    # y viewed as [b, (c s1), (h w)] -- partition p = 2c + s1
    yr = y.rearrange("b c (s hh) w -> b (c s) (hh w)", s=2)

    with tile.TileContext(nc) as tc:
        with (
            tc.tile_pool(name="tin", bufs=3) as pin,
            tc.tile_pool(name="tout", bufs=2) as pout,
        ):
            for b in range(_BPC):
                T = pin.tile([128, _FD], mybir.dt.float32)
                for s1 in range(2):
                    for s2 in range(2):
                        nc.sync.dma_start(
                            out=T[s1::2, s2 * _HW : (s2 + 1) * _HW],
                            in_=xr[b, s1, s2],
                        )
                S = pout.tile([128, _FD], mybir.dt.float32)
                # One 4D-AP copy does the whole interleave:
                # S[p, h*128 + s2*64 + w] <- T[p, s2*4096 + h*64 + w]
                src = T[:, :].rearrange("p (s2 h w) -> p h s2 w", s2=2, w=_W)
                dst = S.rearrange("p (h s2 w) -> p h s2 w", s2=2, w=_W)
                nc.vector.tensor_copy(out=dst, in_=src)
                # Stores ride the second HWDGE ring (ACT); loads own the SP
                # ring, so a store waiting on copies can't head-of-line
                # block the next iteration's loads.
                nc.scalar.dma_start(out=yr[b], in_=S[:, :])
    _split_multiwaits(nc, mybir)
    return nc


def kernel(x: np.ndarray) -> np.ndarray:
    from concourse.bass_utils import run_bass_kernel_spmd

    if "nc" not in _cache:
        _cache["nc"] = _build()
    nc = _cache["nc"]

    x = np.ascontiguousarray(np.asarray(x), dtype=np.float32)
    in_maps = [{"x": x[i * _BPC : (i + 1) * _BPC]} for i in range(_NCORES)]
    res = run_bass_kernel_spmd(nc, in_maps, list(range(_NCORES)))
    return np.concatenate([res.results[i]["y"] for i in range(_NCORES)], axis=0)
